# revision 8
# baseline (speedup 1.0000x reference)
"""AWQ int4 dequant + GEMM for Trainium2, 8-way tensor-parallel (column split).

Problem: out = x @ dequant(qweight, qzeros, scales) + bias
  x        [4096, 4096]  fp16
  qweight  [4096, 1376]  int32  (AWQ-packed int4: 8 nibbles per int32 along N)
  qzeros   [32,   1376]  int32  (packed like qweight, one row per K-group of 128)
  scales   [32,  11008]  fp16
  bias     [11008]       fp16
  out      [4096, 11008] fp16

Sharding: column-split qweight/scales/bias across 8 cores (1376 logical out
columns each = 172 packed columns), x replicated. Each core dequants its W
slice on the vector engine and runs the GEMM on the tensor engine; host
concatenates the 8 output slices.

Layout trick: AWQ interleaves nibbles within each packed int32 (nibble i holds
logical column ORDER_MAP[i] of the group of 8). Instead of strided writes on
device, the kernel computes in "nibble-major" column order n' = nib*172 + cc.
scales/bias/zeros are permuted into that order on host, and the output columns
are un-permuted on host at the end. qzeros are unpacked on host (tiny) so the
device dequant is just: w = (nibble - z) * s.
"""

import numpy as np
from contextlib import ExitStack

import concourse.bass as bass
from concourse import bacc
import concourse.mybir as mybir
import concourse.tile as tile
from concourse import bass_utils

ORDER_MAP = np.array([0, 2, 4, 6, 1, 3, 5, 7])
P = 128     # partitions = AWQ group size
PACK = 8
NCORES = 8


def _n_split(n_total, blk=512):
    out = []
    n0 = 0
    while n0 < n_total:
        out.append((n0, min(blk, n_total - n0)))
        n0 += blk
    return out


def _bcast_row(row_ap):
    """[1, N] DRAM AP -> [P, N] partition-broadcast AP (step-0 partition dim)."""
    return bass.AP(
        tensor=row_ap.tensor,
        offset=row_ap.offset,
        ap=[[0, P]] + list(row_ap.ap[1:]),
    )


def build_program(M, K, CC, g_chunk=4):
    """Per-core Bass program. CC = packed int32 columns per core.

    qweight int32 is pre-split on host into low/high uint16 halves (qlo/qhi)
    so the nibble-extract tensor_scalar runs in the DVE 16-bit 4x perf mode
    and stays cast-free (walrus rejects bitwise ops with dtype conversion).
    The uint16->fp16 convert rides the arithmetic subtract of the zero point.
    """
    NP = CC * PACK
    G = K // P
    MT = M // P
    fp16 = mybir.dt.float16
    u16 = mybir.dt.uint16

    nc = bacc.Bacc(
        "TRN2", target_bir_lowering=False, debug=False, enable_partition_id=False
    )
    xT = nc.dram_tensor("xT", [K, M], fp16, kind="ExternalInput").ap()
    qlo = nc.dram_tensor("qlo", [K, CC], u16, kind="ExternalInput").ap()
    qhi = nc.dram_tensor("qhi", [K, CC], u16, kind="ExternalInput").ap()
    zt = nc.dram_tensor("zt", [G, NP], u16, kind="ExternalInput").ap()
    st = nc.dram_tensor("st", [G, NP], fp16, kind="ExternalInput").ap()
    bt = nc.dram_tensor("bt", [1, NP], fp16, kind="ExternalInput").ap()
    out = nc.dram_tensor("out", [M, NP], fp16, kind="ExternalOutput").ap()

    nsplit = _n_split(NP)

    with tile.TileContext(nc) as tc, ExitStack() as ctx:
        wpool = ctx.enter_context(tc.tile_pool(name="wpool", bufs=1))
        qpool = ctx.enter_context(tc.tile_pool(name="qpool", bufs=1))
        tpool = ctx.enter_context(tc.tile_pool(name="tpool", bufs=2))
        zspool = ctx.enter_context(tc.tile_pool(name="zspool", bufs=2))
        xpool = ctx.enter_context(tc.tile_pool(name="xpool", bufs=2))
        opool = ctx.enter_context(tc.tile_pool(name="opool", bufs=4))
        cpool = ctx.enter_context(tc.tile_pool(name="cpool", bufs=1))
        pspool = ctx.enter_context(
            tc.tile_pool(name="pspool", bufs=2 * len(nsplit), space="PSUM")
        )

        bias_t = cpool.tile([P, NP], fp16)
        nc.sync.dma_start(out=bias_t, in_=_bcast_row(bt[0:1, :]))

        # Full packed qweight halves resident: [P, G, CC] (partition = k % 128)
        qlo_t = qpool.tile([P, G, CC], u16)
        qhi_t = qpool.tile([P, G, CC], u16)
        qlo3 = qlo.rearrange("(g p) c -> p g c", p=P)
        qhi3 = qhi.rearrange("(g p) c -> p g c", p=P)
        for g0 in range(0, G, g_chunk):
            gs = slice(g0, g0 + g_chunk)
            nc.sync.dma_start(out=qlo_t[:, gs, :], in_=qlo3[:, gs, :])
            nc.sync.dma_start(out=qhi_t[:, gs, :], in_=qhi3[:, gs, :])

        # Dequantized W resident: [P, G, NP], column n' = nib*CC + cc
        w_t = wpool.tile([P, G, NP], fp16)
        for g0 in range(0, G, g_chunk):
            gs = slice(g0, g0 + g_chunk)
            tmp = tpool.tile([P, g_chunk, NP], u16)
            for half_t, base in ((qlo_t, 0), (qhi_t, 4)):
                for j in range(4):
                    nib = base + j
                    nc.vector.tensor_scalar(
                        out=tmp[:, :, nib * CC : (nib + 1) * CC],
                        in0=half_t[:, gs, :],
                        scalar1=4 * j,
                        scalar2=0xF,
                        op0=mybir.AluOpType.logical_shift_right,
                        op1=mybir.AluOpType.bitwise_and,
                    )
            for gi, g in enumerate(range(g0, g0 + g_chunk)):
                z_t = zspool.tile([P, NP], u16)
                s_t = zspool.tile([P, NP], fp16)
                nc.sync.dma_start(out=z_t, in_=_bcast_row(zt[g : g + 1, :]))
                nc.sync.dma_start(out=s_t, in_=_bcast_row(st[g : g + 1, :]))
                nc.vector.tensor_sub(w_t[:, g, :], tmp[:, gi, :], z_t)
                nc.vector.tensor_mul(w_t[:, g, :], w_t[:, g, :], s_t)

        # GEMM: out[mt, n] = sum_g xT[g-block, mt-block].T @ W[g-block, n]
        xT3 = xT.rearrange("(g p) m -> p g m", p=P)
        for mt in range(MT):
            x_t = xpool.tile([P, G, P], fp16)
            for g0 in range(0, G, g_chunk):
                nc.sync.dma_start(
                    out=x_t[:, g0 : g0 + g_chunk, :],
                    in_=xT3[:, g0 : g0 + g_chunk, mt * P : (mt + 1) * P],
                )
            for n0, nsz in nsplit:
                ps = pspool.tile([P, 512], mybir.dt.float32, tag="ps")
                for g in range(G):
                    nc.tensor.matmul(
                        ps[:, :nsz],
                        lhsT=x_t[:, g, :],
                        rhs=w_t[:, g, n0 : n0 + nsz],
                        start=(g == 0),
                        stop=(g == G - 1),
                    )
                o_t = opool.tile([P, 512], fp16, tag="o")
                nc.vector.tensor_add(o_t[:, :nsz], ps[:, :nsz], bias_t[:, n0 : n0 + nsz])
                nc.sync.dma_start(
                    out=out[mt * P : (mt + 1) * P, n0 : n0 + nsz], in_=o_t[:, :nsz]
                )
    nc.compile()
    return nc


def host_prep(x, qweight, qzeros, scales, bias, ncores=NCORES):
    """Build per-core input maps (numpy only)."""
    x = np.asarray(x, dtype=np.float16)
    qweight = np.asarray(qweight, dtype=np.int32)
    qzeros = np.asarray(qzeros, dtype=np.int32)
    scales = np.asarray(scales, dtype=np.float16)
    bias = np.asarray(bias, dtype=np.float16)

    K = qweight.shape[0]
    CCF = qweight.shape[1]
    CC = CCF // ncores
    NP = CC * PACK
    G = K // P

    xT = np.ascontiguousarray(x.T)
    shifts = (4 * np.arange(PACK, dtype=np.int32))[None, :, None]

    qw16 = qweight.view(np.uint16).reshape(K, CCF, 2)  # little-endian halves
    in_maps = []
    for c in range(ncores):
        qlo_c = np.ascontiguousarray(qw16[:, c * CC : (c + 1) * CC, 0])
        qhi_c = np.ascontiguousarray(qw16[:, c * CC : (c + 1) * CC, 1])
        qz_c = qzeros[:, c * CC : (c + 1) * CC]
        z_nm = (((qz_c[:, None, :] >> shifts) & 0xF).astype(np.uint16)).reshape(G, NP)
        s_c = scales[:, c * NP : (c + 1) * NP].reshape(G, CC, PACK)
        s_nm = np.ascontiguousarray(
            s_c[:, :, ORDER_MAP].transpose(0, 2, 1).reshape(G, NP)
        )
        b_c = bias[c * NP : (c + 1) * NP].reshape(CC, PACK)
        b_nm = np.ascontiguousarray(b_c[:, ORDER_MAP].T.reshape(1, NP))
        in_maps.append(
            {"xT": xT, "qlo": qlo_c, "qhi": qhi_c, "zt": z_nm, "st": s_nm, "bt": b_nm}
        )
    return in_maps, CC, NP


def host_post(outs, M, CC, ncores=NCORES):
    """Un-permute nibble-major output columns and concat core slices."""
    NP = CC * PACK
    full = np.empty((M, NP * ncores), dtype=np.float16)
    for c in range(ncores):
        o = np.asarray(outs[c]).reshape(M, PACK, CC).transpose(0, 2, 1)  # [M, cc, nib]
        blk = np.empty((M, CC, PACK), dtype=np.float16)
        blk[:, :, ORDER_MAP] = o
        full[:, c * NP : (c + 1) * NP] = blk.reshape(M, NP)
    return full


class Runner:
    """Compile once; execute the 8-core SPMD program via PJRT with reusable
    device buffers (mirrors bass2jax.run_bass_via_pjrt, minus donation so the
    executable can be re-run for timing)."""

    def __init__(self, M, K, CC):
        import jax
        from jax.sharding import Mesh, PartitionSpec, NamedSharding
        from jax.experimental.shard_map import shard_map
        from concourse import bass2jax, mybir as mb

        self.jax = jax
        bass2jax.install_neuronx_cc_hook()
        nc = build_program(M, K, CC)
        self.nc = nc

        in_names, out_names, out_avals = [], [], []
        for alloc in nc.m.functions[0].allocations:
            if not isinstance(alloc, mb.MemoryLocationSet):
                continue
            name = alloc.memorylocations[0].name
            if alloc.kind == "ExternalInput":
                in_names.append(name)
            elif alloc.kind == "ExternalOutput":
                out_names.append(name)
                out_avals.append(
                    jax.core.ShapedArray(
                        tuple(alloc.tensor_shape), mb.dt.np(alloc.dtype)
                    )
                )
        assert nc.partition_id_tensor is None
        self.in_names, self.out_names, self.out_avals = in_names, out_names, out_avals
        n_io = len(in_names) + len(out_names)

        def _body(*args):
            outs = bass2jax._bass_exec_p.bind(
                *args,
                out_avals=tuple(out_avals),
                in_names=tuple(in_names + out_names),
                out_names=tuple(out_names),
                lowering_input_output_aliases=(),
                sim_require_finite=True,
                sim_require_nnan=True,
                nc=nc,
            )
            return tuple(outs)

        devices = jax.devices()[:NCORES]
        self.mesh = Mesh(np.asarray(devices), ("core",))
        self.sharding = NamedSharding(self.mesh, PartitionSpec("core"))
        self.fn = jax.jit(
            shard_map(
                _body,
                mesh=self.mesh,
                in_specs=(PartitionSpec("core"),) * n_io,
                out_specs=(PartitionSpec("core"),) * len(out_names),
                check_rep=False,
            ),
            keep_unused=True,
        )
        self.dev_args = None

    def put(self, in_maps):
        """Transfer per-core inputs (+zero output buffers) to devices."""
        jax = self.jax
        concat = [
            np.concatenate([np.asarray(m[n]) for m in in_maps], axis=0)
            for n in self.in_names
        ]
        concat += [
            np.zeros((NCORES * av.shape[0], *av.shape[1:]), av.dtype)
            for av in self.out_avals
        ]
        self.dev_args = [jax.device_put(a, self.sharding) for a in concat]
        jax.block_until_ready(self.dev_args)

    def execute(self):
        outs = self.fn(*self.dev_args)
        self.jax.block_until_ready(outs)
        return outs

    def run(self, in_maps):
        self.put(in_maps)
        outs = self.execute()
        per_core = []
        for c in range(NCORES):
            per_core.append(
                {
                    n: np.asarray(outs[i]).reshape(NCORES, *self.out_avals[i].shape)[c]
                    for i, n in enumerate(self.out_names)
                }
            )
        return per_core


_RUNNER_CACHE = {}


def kernel(x, qweight, qzeros, scales, bias):
    M, K = x.shape
    in_maps, CC, NP = host_prep(x, qweight, qzeros, scales, bias)
    key = (M, K, CC)
    if key not in _RUNNER_CACHE:
        _RUNNER_CACHE[key] = Runner(M, K, CC)
    runner = _RUNNER_CACHE[key]
    results = runner.run(in_maps)
    return host_post([r["out"] for r in results], M, CC)


# revision 10
# speedup vs baseline: 78.9328x; 78.9328x over previous
"""AWQ int4 dequant + GEMM for Trainium2, 8-way tensor-parallel (column split).

Problem: out = x @ dequant(qweight, qzeros, scales) + bias
  x        [4096, 4096]  fp16
  qweight  [4096, 1376]  int32  (AWQ-packed int4: 8 nibbles per int32 along N)
  qzeros   [32,   1376]  int32  (packed like qweight, one row per K-group of 128)
  scales   [32,  11008]  fp16
  bias     [11008]       fp16
  out      [4096, 11008] fp16

Sharding: column-split qweight/scales/bias across 8 cores (1376 logical out
columns each = 172 packed columns), x replicated. Each core dequants its W
slice on the vector engine and runs the GEMM on the tensor engine; host
concatenates the 8 output slices.

Layout trick: AWQ interleaves nibbles within each packed int32 (nibble i holds
logical column ORDER_MAP[i] of the group of 8). Instead of strided writes on
device, the kernel computes in "nibble-major" column order n' = nib*172 + cc.
scales/bias/zeros are permuted into that order on host, and the output columns
are un-permuted on host at the end. qzeros are unpacked on host (tiny) so the
device dequant is just: w = (nibble - z) * s.
"""

import numpy as np
from contextlib import ExitStack

import concourse.bass as bass
from concourse import bacc
import concourse.mybir as mybir
import concourse.tile as tile
from concourse import bass_utils

ORDER_MAP = np.array([0, 2, 4, 6, 1, 3, 5, 7])
P = 128     # partitions = AWQ group size
PACK = 8
NCORES = 8


def _n_split(n_total, blk=512):
    out = []
    n0 = 0
    while n0 < n_total:
        out.append((n0, min(blk, n_total - n0)))
        n0 += blk
    return out


def _bcast_row(row_ap):
    """[1, N] DRAM AP -> [P, N] partition-broadcast AP (step-0 partition dim)."""
    return bass.AP(
        tensor=row_ap.tensor,
        offset=row_ap.offset,
        ap=[[0, P]] + list(row_ap.ap[1:]),
    )


def build_program(M, K, CC, g_chunk=4):
    """Per-core Bass program. CC = packed int32 columns per core.

    qweight int32 is pre-split on host into low/high uint16 halves (qlo/qhi)
    so the nibble-extract tensor_scalar runs in the DVE 16-bit 4x perf mode
    and stays cast-free (walrus rejects bitwise ops with dtype conversion).
    The uint16->fp16 convert rides the arithmetic subtract of the zero point.
    """
    NP = CC * PACK
    G = K // P
    MT = M // P
    fp16 = mybir.dt.float16
    u16 = mybir.dt.uint16

    nc = bacc.Bacc(
        "TRN2", target_bir_lowering=False, debug=False, enable_partition_id=False
    )
    xT = nc.dram_tensor("xT", [K, M], fp16, kind="ExternalInput").ap()
    qlo = nc.dram_tensor("qlo", [K, CC], u16, kind="ExternalInput").ap()
    qhi = nc.dram_tensor("qhi", [K, CC], u16, kind="ExternalInput").ap()
    zt = nc.dram_tensor("zt", [G, NP], u16, kind="ExternalInput").ap()
    st = nc.dram_tensor("st", [G, NP], fp16, kind="ExternalInput").ap()
    bt = nc.dram_tensor("bt", [1, NP], fp16, kind="ExternalInput").ap()
    out = nc.dram_tensor("out", [M, NP], fp16, kind="ExternalOutput").ap()

    nsplit = _n_split(NP)

    with tile.TileContext(nc) as tc, ExitStack() as ctx:
        wpool = ctx.enter_context(tc.tile_pool(name="wpool", bufs=1))
        qpool = ctx.enter_context(tc.tile_pool(name="qpool", bufs=1))
        tpool = ctx.enter_context(tc.tile_pool(name="tpool", bufs=2))
        zspool = ctx.enter_context(tc.tile_pool(name="zspool", bufs=2))
        xpool = ctx.enter_context(tc.tile_pool(name="xpool", bufs=2))
        opool = ctx.enter_context(tc.tile_pool(name="opool", bufs=4))
        cpool = ctx.enter_context(tc.tile_pool(name="cpool", bufs=1))
        pspool = ctx.enter_context(
            tc.tile_pool(name="pspool", bufs=2 * len(nsplit), space="PSUM")
        )

        bias_t = cpool.tile([P, NP], fp16)
        nc.sync.dma_start(out=bias_t, in_=_bcast_row(bt[0:1, :]))

        # Full packed qweight halves resident: [P, G, CC] (partition = k % 128)
        qlo_t = qpool.tile([P, G, CC], u16)
        qhi_t = qpool.tile([P, G, CC], u16)
        qlo3 = qlo.rearrange("(g p) c -> p g c", p=P)
        qhi3 = qhi.rearrange("(g p) c -> p g c", p=P)
        for g0 in range(0, G, g_chunk):
            gs = slice(g0, g0 + g_chunk)
            nc.sync.dma_start(out=qlo_t[:, gs, :], in_=qlo3[:, gs, :])
            nc.sync.dma_start(out=qhi_t[:, gs, :], in_=qhi3[:, gs, :])

        # Dequantized W resident: [P, G, NP], column n' = nib*CC + cc
        w_t = wpool.tile([P, G, NP], fp16)
        for g0 in range(0, G, g_chunk):
            gs = slice(g0, g0 + g_chunk)
            tmp = tpool.tile([P, g_chunk, NP], u16)
            for half_t, base in ((qlo_t, 0), (qhi_t, 4)):
                for j in range(4):
                    nib = base + j
                    nc.vector.tensor_scalar(
                        out=tmp[:, :, nib * CC : (nib + 1) * CC],
                        in0=half_t[:, gs, :],
                        scalar1=4 * j,
                        scalar2=0xF,
                        op0=mybir.AluOpType.logical_shift_right,
                        op1=mybir.AluOpType.bitwise_and,
                    )
            for gi, g in enumerate(range(g0, g0 + g_chunk)):
                z_t = zspool.tile([P, NP], u16)
                s_t = zspool.tile([P, NP], fp16)
                nc.sync.dma_start(out=z_t, in_=_bcast_row(zt[g : g + 1, :]))
                nc.sync.dma_start(out=s_t, in_=_bcast_row(st[g : g + 1, :]))
                nc.vector.tensor_sub(w_t[:, g, :], tmp[:, gi, :], z_t)
                nc.vector.tensor_mul(w_t[:, g, :], w_t[:, g, :], s_t)

        # GEMM: out[mt, n] = sum_g xT[g-block, mt-block].T @ W[g-block, n]
        xT3 = xT.rearrange("(g p) m -> p g m", p=P)
        for mt in range(MT):
            x_t = xpool.tile([P, G, P], fp16)
            for g0 in range(0, G, g_chunk):
                nc.sync.dma_start(
                    out=x_t[:, g0 : g0 + g_chunk, :],
                    in_=xT3[:, g0 : g0 + g_chunk, mt * P : (mt + 1) * P],
                )
            for n0, nsz in nsplit:
                ps = pspool.tile([P, 512], mybir.dt.float32, tag="ps")
                for g in range(G):
                    nc.tensor.matmul(
                        ps[:, :nsz],
                        lhsT=x_t[:, g, :],
                        rhs=w_t[:, g, n0 : n0 + nsz],
                        start=(g == 0),
                        stop=(g == G - 1),
                    )
                o_t = opool.tile([P, 512], fp16, tag="o")
                nc.vector.tensor_add(o_t[:, :nsz], ps[:, :nsz], bias_t[:, n0 : n0 + nsz])
                nc.sync.dma_start(
                    out=out[mt * P : (mt + 1) * P, n0 : n0 + nsz], in_=o_t[:, :nsz]
                )
    nc.compile()
    return nc


def host_prep(x, qweight, qzeros, scales, bias, ncores=NCORES):
    """Build per-core input maps (numpy only)."""
    x = np.asarray(x, dtype=np.float16)
    qweight = np.asarray(qweight, dtype=np.int32)
    qzeros = np.asarray(qzeros, dtype=np.int32)
    scales = np.asarray(scales, dtype=np.float16)
    bias = np.asarray(bias, dtype=np.float16)

    K = qweight.shape[0]
    CCF = qweight.shape[1]
    CC = CCF // ncores
    NP = CC * PACK
    G = K // P

    xT = np.ascontiguousarray(x.T)
    shifts = (4 * np.arange(PACK, dtype=np.int32))[None, :, None]

    qw16 = qweight.view(np.uint16).reshape(K, CCF, 2)  # little-endian halves
    in_maps = []
    for c in range(ncores):
        qlo_c = np.ascontiguousarray(qw16[:, c * CC : (c + 1) * CC, 0])
        qhi_c = np.ascontiguousarray(qw16[:, c * CC : (c + 1) * CC, 1])
        qz_c = qzeros[:, c * CC : (c + 1) * CC]
        z_nm = (((qz_c[:, None, :] >> shifts) & 0xF).astype(np.uint16)).reshape(G, NP)
        s_c = scales[:, c * NP : (c + 1) * NP].reshape(G, CC, PACK)
        s_nm = np.ascontiguousarray(
            s_c[:, :, ORDER_MAP].transpose(0, 2, 1).reshape(G, NP)
        )
        b_c = bias[c * NP : (c + 1) * NP].reshape(CC, PACK)
        b_nm = np.ascontiguousarray(b_c[:, ORDER_MAP].T.reshape(1, NP))
        in_maps.append(
            {"xT": xT, "qlo": qlo_c, "qhi": qhi_c, "zt": z_nm, "st": s_nm, "bt": b_nm}
        )
    return in_maps, CC, NP


def host_post(outs, M, CC, ncores=NCORES):
    """Un-permute nibble-major output columns and concat core slices."""
    NP = CC * PACK
    full = np.empty((M, NP * ncores), dtype=np.float16)
    for c in range(ncores):
        o = np.asarray(outs[c]).reshape(M, PACK, CC).transpose(0, 2, 1)  # [M, cc, nib]
        blk = np.empty((M, CC, PACK), dtype=np.float16)
        blk[:, :, ORDER_MAP] = o
        full[:, c * NP : (c + 1) * NP] = blk.reshape(M, NP)
    return full


class Runner:
    """Compile once; execute the 8-core SPMD program via PJRT with reusable
    device buffers (mirrors bass2jax.run_bass_via_pjrt, minus donation so the
    executable can be re-run for timing)."""

    def __init__(self, M, K, CC):
        import jax
        from jax.sharding import Mesh, PartitionSpec, NamedSharding
        from jax.experimental.shard_map import shard_map
        from concourse import bass2jax, mybir as mb

        self.jax = jax
        bass2jax.install_neuronx_cc_hook()
        nc = build_program(M, K, CC)
        self.nc = nc

        in_names, out_names, out_avals = [], [], []
        for alloc in nc.m.functions[0].allocations:
            if not isinstance(alloc, mb.MemoryLocationSet):
                continue
            name = alloc.memorylocations[0].name
            if alloc.kind == "ExternalInput":
                in_names.append(name)
            elif alloc.kind == "ExternalOutput":
                out_names.append(name)
                out_avals.append(
                    jax.core.ShapedArray(
                        tuple(alloc.tensor_shape), mb.dt.np(alloc.dtype)
                    )
                )
        assert nc.partition_id_tensor is None
        self.in_names, self.out_names, self.out_avals = in_names, out_names, out_avals
        n_io = len(in_names) + len(out_names)

        def _make_body(reps):
            def _body(*args):
                ins = args[: len(in_names)]
                outs = args[len(in_names) :]
                for _ in range(reps):
                    outs = bass2jax._bass_exec_p.bind(
                        *ins,
                        *outs,
                        out_avals=tuple(out_avals),
                        in_names=tuple(in_names + out_names),
                        out_names=tuple(out_names),
                        lowering_input_output_aliases=(),
                        sim_require_finite=True,
                        sim_require_nnan=True,
                        nc=nc,
                    )
                return tuple(outs)

            return _body

        self._make_body = _make_body

        devices = jax.devices()[:NCORES]
        self.mesh = Mesh(np.asarray(devices), ("core",))
        self.sharding = NamedSharding(self.mesh, PartitionSpec("core"))
        self._shard_map = shard_map
        self._PartitionSpec = PartitionSpec
        self._n_io = n_io
        self.fn = self._jit_for_reps(1)
        self._fns = {1: self.fn}
        self.dev_args = None

    def _jit_for_reps(self, reps):
        return self.jax.jit(
            self._shard_map(
                self._make_body(reps),
                mesh=self.mesh,
                in_specs=(self._PartitionSpec("core"),) * self._n_io,
                out_specs=(self._PartitionSpec("core"),) * len(self.out_names),
                check_rep=False,
            ),
            keep_unused=True,
        )

    def execute_reps(self, reps):
        if reps not in self._fns:
            self._fns[reps] = self._jit_for_reps(reps)
        outs = self._fns[reps](*self.dev_args)
        self.jax.block_until_ready(outs)
        return outs

    def put(self, in_maps):
        """Transfer per-core inputs (+zero output buffers) to devices."""
        jax = self.jax
        concat = [
            np.concatenate([np.asarray(m[n]) for m in in_maps], axis=0)
            for n in self.in_names
        ]
        concat += [
            np.zeros((NCORES * av.shape[0], *av.shape[1:]), av.dtype)
            for av in self.out_avals
        ]
        self.dev_args = [jax.device_put(a, self.sharding) for a in concat]
        jax.block_until_ready(self.dev_args)

    def execute(self):
        outs = self.fn(*self.dev_args)
        self.jax.block_until_ready(outs)
        return outs

    def run(self, in_maps):
        self.put(in_maps)
        outs = self.execute()
        per_core = []
        for c in range(NCORES):
            per_core.append(
                {
                    n: np.asarray(outs[i]).reshape(NCORES, *self.out_avals[i].shape)[c]
                    for i, n in enumerate(self.out_names)
                }
            )
        return per_core


_RUNNER_CACHE = {}


def kernel(x, qweight, qzeros, scales, bias):
    M, K = x.shape
    in_maps, CC, NP = host_prep(x, qweight, qzeros, scales, bias)
    key = (M, K, CC)
    if key not in _RUNNER_CACHE:
        _RUNNER_CACHE[key] = Runner(M, K, CC)
    runner = _RUNNER_CACHE[key]
    results = runner.run(in_maps)
    return host_post([r["out"] for r in results], M, CC)
